# revision 4
# baseline (speedup 1.0000x reference)
"""Trainium2 Bass kernel for nn_CustomModel_21019569946955 (pendulum Lyapunov loss).

Data-parallel over 8 NeuronCores: each core processes B/8 = 8192 samples with
replicated MLP weights. Per core, a feature-major (activations transposed)
fp32r pipeline:

  phase A (fwd):  h1 = tanh(W1^T x^T); h2 = tanh(W2^T h1); [y_pred; V] = W3^T h2
                  g2h = (1 - h2^2) * W3[:,1]  (spilled to DRAM per tile)
                  (W2 is PE-transposed into DRAM first)
  phase C (bwd):  g1h = (1 - h1^2) * (W2 g2h)  (h1 recomputed on demand)
                  dVdx = W1 g1h
  final:          batch-major pendulum ODE + penalties + partial sums for the
                  scalar custom_loss (combined on host: pure data-parallel mean).
"""
import numpy as np
import concourse.bass as bass
import concourse.tile as tile
from concourse import bacc, mybir
from concourse.bass_utils import run_bass_kernel_spmd
from concourse.masks import make_identity

F32 = mybir.dt.float32
F32R = mybir.dt.float32r
AF = mybir.ActivationFunctionType
ALU = mybir.AluOpType

# problem constants (hardcoded from the reference)
G = 9.8
L, I_, MB, MC, AT, AR = 0.3, 2.0, 1.0, 3.0, 0.2, 0.2
C1 = L * MB            # 0.3
C2 = I_ + L * L * MB   # 2.09
C3 = MB + MC           # 4.0
PEN = 10000.0
ALPHA = 0.1
EPS = 1e-7
C1SQ = C1 * C1
C2C3 = C2 * C3

B, H, D = 65536, 2048, 4
NCORES = 8
BC = B // NCORES        # 8192 samples per core
N = 512                 # batch-chunk (moving free dim)
CH = BC // N            # 16 chunks
KT = H // 128           # 16 feature tiles
FB = BC // 128          # 64 samples per partition in the final stage

# fp32 round-to-nearest-int trick + Cody-Waite 2pi for sin/cos range reduction
RC = float(1.5 * 2 ** 23)
INV2PI = float(1.0 / (2.0 * np.pi))
TWOPI_HI = float(np.float32(2.0 * np.pi))
TWOPI_LO = float(2.0 * np.pi - np.float64(np.float32(2.0 * np.pi)))
HALFPI = float(np.pi / 2)

_NC_CACHE = {}


def build():
    nc = bacc.Bacc("TRN2", target_bir_lowering=False, debug=False)

    xd = nc.declare_dram_parameter("x", [BC, D], F32, isOutput=False)
    yd = nc.declare_dram_parameter("y", [BC], F32, isOutput=False)
    W1d = nc.declare_dram_parameter("W1", [D, H], F32, isOutput=False)
    b1d = nc.declare_dram_parameter("b1", [H], F32, isOutput=False)
    W2d = nc.declare_dram_parameter("W2", [H, H], F32, isOutput=False)
    b2d = nc.declare_dram_parameter("b2", [H], F32, isOutput=False)
    W3d = nc.declare_dram_parameter("W3", [H, 2], F32, isOutput=False)
    b3d = nc.declare_dram_parameter("b3", [2], F32, isOutput=False)

    loss_out = nc.declare_dram_parameter("loss_pen", [BC], F32, isOutput=True)
    part_out = nc.declare_dram_parameter("partials", [128, 2], F32, isOutput=True)

    with tile.TileContext(nc) as tc:
        with tc.tile_pool(name="dram", bufs=1, space="DRAM") as dpool:
            # DRAM scratch (tile-pool so Tile tracks write->read deps)
            w2t_d = dpool.tile([KT, 128, H], F32R, tag="w2t_d", name="w2t_d")
            g2h_d = dpool.tile([CH, 128, KT, N], F32R, tag="g2h_d", name="g2h_d")
            yv_d = dpool.tile([2, BC], F32, tag="yv_d", name="yv_d")
            dv_d = dpool.tile([D, BC], F32, tag="dv_d", name="dv_d")

            with tc.tile_pool(name="wpool", bufs=1) as wpool, \
                 tc.tile_pool(name="small", bufs=1) as small:

                # ---- weights / constants (alive through both phases) ----
                w2sb = wpool.tile([128, KT, H], F32R, tag="w2", name="w2sb")
                nc.gpsimd.dma_start(w2sb[:], W2d.rearrange("(k p) c -> p k c", p=128))

                w1sb = small.tile([D, KT, 128], F32R, tag="w1", name="w1sb")
                nc.gpsimd.dma_start(w1sb[:], W1d.rearrange("d (k c) -> d k c", c=128))
                w1t = small.tile([128, KT, D], F32R, tag="w1t", name="w1t")
                for k in range(KT):
                    nc.gpsimd.dma_start(
                        w1t[:, k], W1d[:, k * 128:(k + 1) * 128].rearrange("d p -> p d"))
                w3sb = small.tile([128, KT, 2], F32R, tag="w3", name="w3sb")
                nc.gpsimd.dma_start(w3sb[:], W3d.rearrange("(k p) j -> p k j", p=128))
                b1c = small.tile([128, KT], F32, tag="b1c", name="b1c")
                nc.sync.dma_start(b1c[:], b1d.rearrange("(k p) -> p k", p=128))
                b2c = small.tile([128, KT], F32, tag="b2c", name="b2c")
                nc.sync.dma_start(b2c[:], b2d.rearrange("(k p) -> p k", p=128))
                b3c = small.tile([2, 1], F32, tag="b3c", name="b3c")
                nc.sync.dma_start(b3c[:], b3d.rearrange("(p o) -> p o", o=1))
                w3c1 = small.tile([128, KT, 1], F32, tag="w3c1", name="w3c1")
                nc.sync.dma_start(
                    w3c1[:], W3d.rearrange("(k p) j -> p k j", p=128)[:, :, 1:2])
                nw3c1 = small.tile([128, KT, 1], F32, tag="nw3c1", name="nw3c1")
                nc.vector.tensor_scalar_mul(nw3c1[:], w3c1[:], -1.0)

                ident_f = small.tile([128, 128], F32, tag="ident_f", name="ident_f")
                make_identity(nc, ident_f[:])
                ident = small.tile([128, 128], F32R, tag="ident", name="ident")
                nc.vector.tensor_copy(ident[:], ident_f[:])

                with tc.tile_pool(name="big", bufs=1) as big, \
                     tc.tile_pool(name="tmp", bufs=2) as tmp, \
                     tc.tile_pool(name="pp", bufs=2, space="PSUM") as pp, \
                     tc.tile_pool(name="pp1", bufs=1, space="PSUM") as pp1:

                    # ---- W2^T tiles into DRAM via PE transposes ----
                    for k2 in range(KT):
                        for m1 in range(KT):
                            pst = pp.tile([128, 128], F32R, tag="tr", name="pst")
                            nc.tensor.transpose(
                                pst[:], w2sb[:, m1, k2 * 128:(k2 + 1) * 128], ident[:])
                            ot = tmp.tile([128, 128], F32R, tag="trt", name="ot", bufs=3)
                            nc.scalar.copy(ot[:], pst[:])
                            nc.sync.dma_start(
                                w2t_d[k2, :, m1 * 128:(m1 + 1) * 128], ot[:])

                    # ---- phase A: forward over chunks ----
                    for i in range(CH):
                        xt = tmp.tile([D, N], F32R, tag="xt", name="xt")
                        nc.gpsimd.dma_start(
                            xt[:], xd[i * N:(i + 1) * N, :].rearrange("n d -> d n"))

                        h1sb = big.tile([128, KT, N], F32R, tag="bigbuf", name="h1sb")
                        for m1 in range(KT):
                            ps = pp.tile([128, N], F32, tag="h1x", name="h1ps")
                            nc.tensor.matmul(ps[:], w1sb[:, m1], xt[:],
                                             start=True, stop=True)
                            nc.scalar.activation(h1sb[:, m1], ps[:], AF.Tanh,
                                                 bias=b1c[:, m1:m1 + 1])

                        yvp = pp1.tile([2, N], F32, tag="yvdv", name="yvp")
                        for m2 in range(KT):
                            ps = pp.tile([128, N], F32, tag="mmacc", name="h2ps")
                            for k in range(KT):
                                nc.tensor.matmul(
                                    ps[:], w2sb[:, k, m2 * 128:(m2 + 1) * 128],
                                    h1sb[:, k], start=(k == 0), stop=(k == KT - 1))
                            h2t = tmp.tile([128, N], F32R, tag="actt", name="h2t")
                            nc.scalar.activation(h2t[:], ps[:], AF.Tanh,
                                                 bias=b2c[:, m2:m2 + 1])
                            nc.tensor.matmul(yvp[:], w3sb[:, m2], h2t[:],
                                             start=(m2 == 0), stop=(m2 == KT - 1))
                            sq = tmp.tile([128, N], F32, tag="sq", name="sqA")
                            nc.vector.tensor_mul(sq[:], h2t[:], h2t[:])
                            g2t = tmp.tile([128, N], F32R, tag="gout", name="g2t")
                            nc.vector.tensor_scalar(
                                g2t[:], sq[:], nw3c1[:, m2], w3c1[:, m2],
                                ALU.mult, ALU.add)
                            nc.sync.dma_start(g2h_d[i, :, m2, :], g2t[:])

                        yvt = tmp.tile([D, N], F32, tag="evac", name="yvt")
                        nc.vector.tensor_scalar(yvt[:2, :], yvp[:], b3c[:],
                                                None, ALU.add)
                        nc.sync.dma_start(yv_d[:, i * N:(i + 1) * N], yvt[:2, :])

                    # ---- swap W2 -> W2^T in SBUF (same slot, WAR-serialized) ----
                    w2tsb = wpool.tile([128, KT, H], F32R, tag="w2", name="w2tsb")
                    nc.sync.dma_start(w2tsb[:], w2t_d.rearrange("k p c -> p k c"))

                    # ---- phase C: backward over chunks ----
                    for i in range(CH):
                        g2full = big.tile([128, KT, N], F32R, tag="bigbuf",
                                          name="g2full")
                        nc.sync.dma_start(g2full[:], g2h_d[i])
                        xt = tmp.tile([D, N], F32R, tag="xt", name="xtc")
                        nc.gpsimd.dma_start(
                            xt[:], xd[i * N:(i + 1) * N, :].rearrange("n d -> d n"))

                        dvp = pp1.tile([D, N], F32, tag="yvdv", name="dvp")
                        for m1 in range(KT):
                            hps = pp.tile([128, N], F32, tag="h1x", name="hps")
                            nc.tensor.matmul(hps[:], w1sb[:, m1], xt[:],
                                             start=True, stop=True)
                            h1t = tmp.tile([128, N], F32, tag="actt", name="h1t")
                            nc.scalar.activation(h1t[:], hps[:], AF.Tanh,
                                                 bias=b1c[:, m1:m1 + 1])
                            sq = tmp.tile([128, N], F32, tag="sq", name="sqC")
                            nc.vector.tensor_mul(sq[:], h1t[:], h1t[:])

                            gps = pp.tile([128, N], F32, tag="mmacc", name="gps")
                            for k in range(KT):
                                nc.tensor.matmul(
                                    gps[:], w2tsb[:, k, m1 * 128:(m1 + 1) * 128],
                                    g2full[:, k], start=(k == 0), stop=(k == KT - 1))
                            gt = tmp.tile([128, N], F32, tag="gt", name="gt")
                            nc.vector.tensor_mul(gt[:], gps[:], sq[:])
                            g1h = tmp.tile([128, N], F32R, tag="gout", name="g1h")
                            nc.vector.tensor_sub(g1h[:], gps[:], gt[:])
                            nc.tensor.matmul(dvp[:], w1t[:, m1], g1h[:],
                                             start=(m1 == 0), stop=(m1 == KT - 1))

                        dvt = tmp.tile([D, N], F32, tag="evac", name="dvt")
                        nc.vector.tensor_copy(dvt[:], dvp[:])
                        nc.sync.dma_start(dv_d[:, i * N:(i + 1) * N], dvt[:])

                # ---- final stage: batch-major per-sample math ----
                with tc.tile_pool(name="fpool", bufs=1) as fpool:
                    _final_stage(nc, tc, fpool, xd, yd, yv_d, dv_d,
                                 loss_out, part_out)

    nc.compile()
    return nc


def _final_stage(nc, tc, fpool, xd, yd, yv_d, dv_d, loss_out, part_out):
    def plane_from_row(dram_row_ap, tag):
        t = fpool.tile([128, FB], F32, tag=tag, name=tag)
        nc.sync.dma_start(t[:], dram_row_ap.rearrange("(p f) -> p f", p=128))
        return t

    ypred = plane_from_row(yv_d[0], "ypred")
    vpl = plane_from_row(yv_d[1], "vpl")
    dv0 = plane_from_row(dv_d[0], "dv0")
    dv1 = plane_from_row(dv_d[1], "dv1")
    dv2 = plane_from_row(dv_d[2], "dv2")
    dv3 = plane_from_row(dv_d[3], "dv3")
    ypl = plane_from_row(yd[:], "ypl")

    xpl = fpool.tile([128, FB, D], F32, tag="xpl", name="xpl")
    nc.sync.dma_start(xpl[:], xd.rearrange("(p f) d -> p f d", p=128))
    x2 = xpl[:, :, 1]
    x3 = xpl[:, :, 2]
    x4 = xpl[:, :, 3]

    zc = fpool.tile([128, 1], F32, tag="zc", name="zc")
    nc.vector.memset(zc[:], 0.0)

    def ftile(tag):
        return fpool.tile([128, FB], F32, tag=tag, name=tag)

    def sin_reduced(src_ap, negate, bias, tag):
        # sin(bias + (negate ? -src : src)), range-reduced mod 2pi
        w = ftile(tag + "w")
        nc.vector.tensor_scalar(w[:], src_ap, -1.0 if negate else 1.0, bias,
                                ALU.mult, ALU.add)
        t = ftile(tag + "t")
        nc.vector.tensor_scalar(t[:], w[:], INV2PI, RC, ALU.mult, ALU.add)
        r = ftile(tag + "r")
        nc.vector.tensor_scalar(r[:], t[:], RC, None, ALU.subtract)
        a = ftile(tag + "a")
        nc.vector.scalar_tensor_tensor(a[:], r[:], -TWOPI_HI, w[:], ALU.mult, ALU.add)
        y_ = ftile(tag + "y")
        nc.vector.scalar_tensor_tensor(y_[:], r[:], -TWOPI_LO, a[:], ALU.mult, ALU.add)
        o = ftile(tag + "o")
        nc.scalar.activation(o[:], y_[:], AF.Sin, bias=zc[:])
        return o

    s = sin_reduced(x3, False, 0.0, "s")
    c = sin_reduced(x3, True, HALFPI, "c")     # cos(x) = sin(pi/2 - x)

    f = ftile("f")
    nc.vector.scalar_tensor_tensor(f[:], x2, -AT, ypred[:], ALU.mult, ALU.add)

    u = ftile("u")
    nc.vector.tensor_mul(u[:], c[:], c[:])
    den = ftile("den")
    nc.vector.tensor_scalar(den[:], u[:], -C1SQ, C2C3, ALU.mult, ALU.add)
    rden = ftile("rden")
    nc.vector.reciprocal(rden[:], den[:])

    cs = ftile("cs")
    nc.vector.tensor_mul(cs[:], c[:], s[:])
    x4sq = ftile("x4sq")
    nc.vector.tensor_mul(x4sq[:], x4, x4)
    cx4 = ftile("cx4")
    nc.vector.tensor_mul(cx4[:], c[:], x4)
    sx4sq = ftile("sx4sq")
    nc.vector.tensor_mul(sx4sq[:], s[:], x4sq[:])
    csx4sq = ftile("csx4sq")
    nc.vector.tensor_mul(csx4sq[:], cs[:], x4sq[:])
    cf = ftile("cf")
    nc.vector.tensor_mul(cf[:], c[:], f[:])

    # x2p = (G*C1^2*c*s + C2*f - AR*C1*c*x4 - C1*C2*s*x4^2) / den
    p1 = ftile("p1")
    nc.vector.tensor_scalar(p1[:], f[:], C2, None, ALU.mult)
    nc.vector.scalar_tensor_tensor(p1[:], cs[:], G * C1SQ, p1[:], ALU.mult, ALU.add)
    nc.vector.scalar_tensor_tensor(p1[:], cx4[:], -AR * C1, p1[:], ALU.mult, ALU.add)
    nc.vector.scalar_tensor_tensor(p1[:], sx4sq[:], -C1 * C2, p1[:], ALU.mult, ALU.add)
    x2p = ftile("x2p")
    nc.vector.tensor_mul(x2p[:], p1[:], rden[:])

    # x4p = (G*C1*C3*s + C1*c*f - AR*C3*x4 - C1^2*c*s*x4^2) / den
    p2 = ftile("p2")
    nc.vector.tensor_scalar(p2[:], s[:], G * C1 * C3, None, ALU.mult)
    nc.vector.scalar_tensor_tensor(p2[:], cf[:], C1, p2[:], ALU.mult, ALU.add)
    nc.vector.scalar_tensor_tensor(p2[:], x4, -AR * C3, p2[:], ALU.mult, ALU.add)
    nc.vector.scalar_tensor_tensor(p2[:], csx4sq[:], -C1SQ, p2[:], ALU.mult, ALU.add)
    x4p = ftile("x4p")
    nc.vector.tensor_mul(x4p[:], p2[:], rden[:])

    # Vdot = dV . [x2, x2p, x4, x4p]
    vd = ftile("vd")
    nc.vector.tensor_mul(vd[:], dv0[:], x2)
    t_ = ftile("vt")
    nc.vector.tensor_mul(t_[:], dv1[:], x2p[:])
    nc.vector.tensor_add(vd[:], vd[:], t_[:])
    nc.vector.tensor_mul(t_[:], dv2[:], x4)
    nc.vector.tensor_add(vd[:], vd[:], t_[:])
    nc.vector.tensor_mul(t_[:], dv3[:], x4p[:])
    nc.vector.tensor_add(vd[:], vd[:], t_[:])

    # penalties: PEN*relu(-V) + PEN*relu(Vdot)
    pen = ftile("pen")
    nc.vector.tensor_scalar(pen[:], vpl[:], 0.0, -PEN, ALU.min, ALU.mult)
    pen2 = ftile("pen2")
    nc.vector.tensor_scalar(pen2[:], vd[:], 0.0, PEN, ALU.max, ALU.mult)
    nc.vector.tensor_add(pen[:], pen[:], pen2[:])
    nc.sync.dma_start(loss_out.rearrange("(p f) -> p f", p=128), pen[:])

    # partial sums for custom_loss: sum(d^2), sum((y - y_pred)^2)
    ypc = ftile("ypc")
    nc.vector.tensor_scalar(ypc[:], ypred[:], EPS, None, ALU.max)
    l1 = ftile("l1")
    nc.scalar.activation(l1[:], ypc[:], AF.Ln, bias=1.0)
    yc = ftile("yc")
    nc.vector.tensor_scalar(yc[:], ypl[:], EPS, None, ALU.max)
    l2 = ftile("l2")
    nc.scalar.activation(l2[:], yc[:], AF.Ln, bias=1.0)
    dd = ftile("dd")
    nc.vector.tensor_sub(dd[:], l1[:], l2[:])
    d2s = fpool.tile([128, 1], F32, tag="d2s", name="d2s")
    dtmp = ftile("dtmp")
    nc.scalar.activation(dtmp[:], dd[:], AF.Square, bias=zc[:], accum_out=d2s[:])
    ee = ftile("ee")
    nc.vector.tensor_sub(ee[:], ypl[:], ypred[:])
    es = fpool.tile([128, 1], F32, tag="es", name="es")
    nc.scalar.activation(dtmp[:], ee[:], AF.Square, bias=zc[:], accum_out=es[:])

    parts = fpool.tile([128, 2], F32, tag="parts", name="parts")
    nc.vector.tensor_copy(parts[:, 0:1], d2s[:])
    nc.vector.tensor_copy(parts[:, 1:2], es[:])
    nc.sync.dma_start(part_out[:, :], parts[:])


def kernel(**inputs):
    x = np.ascontiguousarray(inputs["x"], dtype=np.float32)
    y = np.ascontiguousarray(inputs["y"], dtype=np.float32)
    W1 = np.ascontiguousarray(inputs["W1"], dtype=np.float32)
    b1 = np.ascontiguousarray(inputs["b1"], dtype=np.float32)
    W2 = np.ascontiguousarray(inputs["W2"], dtype=np.float32)
    b2 = np.ascontiguousarray(inputs["b2"], dtype=np.float32)
    W3 = np.ascontiguousarray(inputs["W3"], dtype=np.float32)
    b3 = np.ascontiguousarray(inputs["b3"], dtype=np.float32)

    if "nc" not in _NC_CACHE:
        _NC_CACHE["nc"] = build()
    nc = _NC_CACHE["nc"]

    in_maps = []
    for cid in range(NCORES):
        sl = slice(cid * BC, (cid + 1) * BC)
        in_maps.append({
            "x": x[sl], "y": y[sl],
            "W1": W1, "b1": b1, "W2": W2, "b2": b2, "W3": W3, "b3": b3,
        })
    res = run_bass_kernel_spmd(nc, in_maps, list(range(NCORES)))

    loss = np.concatenate([res.results[c]["loss_pen"] for c in range(NCORES)])
    parts = np.stack([res.results[c]["partials"] for c in range(NCORES)])
    sums = parts.astype(np.float64).sum(axis=(0, 1))
    scalar = ALPHA * sums[0] / B + (1.0 - ALPHA) * sums[1] / B
    return (loss + np.float32(scalar)).astype(np.float32)
